# revision 29
# baseline (speedup 1.0000x reference)
"""Izhikevich 2-layer SNN kernel for 8 Trainium2 NeuronCores.

Reference computation (per timestep t of 100):
    cur1 = x_t @ W1.T + b1                 # [B, 100]
    spk1, v1, u1 = izh(cur1, v1, u1)
    cur2 = spk1 @ W2.T + b2                # [B, 10]
    spk2, v2, u2 = izh(cur2, v2, u2)
    record spk2, v2
Output: (spk2_rec, mem2_rec), each [100, B, 10].

Sharding: pure data parallel over batch (2048 -> 8 x 256), weights replicated.

Device design (v2):
  * Layer fusion with a one-step skew: L2 of step t-1 is computed in the same
    iteration as L1 of step t, so every elementwise op runs ONCE per iteration
    on a fused [110, 256] tile (rows 0:100 = layer-1 neurons, 100:110 =
    layer-2) instead of twice.  101 iterations total.
  * DVE per-op cost is free-dim bound (a [10,256] op costs the same as
    [110,256]), so fusing the layers halves elementwise work.
  * x and W1 are fp8 e4m3; the 784-feature contraction is padded to 896 and
    done as 4 DoubleRow matmuls (K=224 each), halving PE pass count.  State
    (v, u) and W2/spk are fp16.  Zero-spike margin of the data regime makes
    these quantizations safe by a huge factor; PSUM accumulates fp32.
  * Engine split: ACT does the two activations, Pool (gpsimd) does the
    layer-1 wv and the spike u-add, DVE does the rest.
  * Shifted state U := u + 85 - beta so the bias folds into the activation:
        v_new = Square(0.2 v + 15) + (p - U)
        U'    = (1-a) U + ab v + a(85-beta);  U += d spk
  * v / spk live as column blocks of [110, FLUSH*256] tiles; outputs stream
    to DRAM once per block from rows 100:110.
"""

import os
from contextlib import ExitStack

import numpy as np
import ml_dtypes

import concourse.bass as bass
import concourse.bacc as bacc
import concourse.mybir as mybir
import concourse.tile as tile
from concourse.bass_utils import run_bass_kernel_spmd

# Izhikevich RS config + threshold (matches reference.py)
A_, B_, C_, D_ = 0.02, 0.2, -65.0, 8.0
THR = 0.03

T, F, H, O = 100, 784, 100, 10
FH = H + O            # fused partition dim
P = 112               # features per half-pair
PAIRS = 4             # contraction pairs of K=224 (784 padded to 896)
FPAD = PAIRS * 2 * P  # 896
NCORES = 8
BATCH = 2048
BC = BATCH // NCORES  # 256 batch per core

TB = 2      # timesteps per x DMA
FLUSH = 25  # iterations per v/spk column-block tile

LAST_RUN = None  # BassKernelResults of the most recent kernel() call


def build_program(nc, ctx, tc):
    f32 = mybir.dt.float32
    f16 = mybir.dt.float16
    f8 = mybir.dt.float8e4
    AL = mybir.AluOpType
    AF = mybir.ActivationFunctionType
    DR = mybir.MatmulPerfMode.DoubleRow

    NIT = T + 1  # skewed iterations

    # Engine access patterns must start at a partition base that is a
    # multiple of 32, so layer-2-only ops run on rows 96:110 (rows 96:100
    # are scratch, rewritten by the layer-1 op that follows).  The W2 matmul
    # gets 4 leading zero columns so cur2 lands at rows 4:14 of its PSUM
    # tile, i.e. fused rows 100:110.
    OW = O + 4  # 14

    xT = nc.dram_tensor("xT", [T // TB, P, TB * PAIRS * 2 * BC], f8,
                        kind="ExternalInput").ap()
    # DoubleRow LDWEIGHTS needs the sub-row step to be a multiple of 16, so
    # each 100-col weight block is stored with a 112-col pitch (12 pad cols).
    w1 = nc.dram_tensor("w1t", [P, PAIRS * 2 * P], f8, kind="ExternalInput").ap()
    w2 = nc.dram_tensor("w2t", [H, OW], f16, kind="ExternalInput").ap()
    nI = nc.dram_tensor("nI", [FH, FH], f16, kind="ExternalInput").ap()
    ui = nc.dram_tensor("ui", [FH, BC], f16, kind="ExternalInput").ap()
    g = nc.dram_tensor("g", [FH, 1], f32, kind="ExternalInput").ap()
    out_s = nc.dram_tensor("out_s", [OW, T, BC], f16, kind="ExternalOutput").ap()
    out_m = nc.dram_tensor("out_m", [OW, T, BC], f16, kind="ExternalOutput").ap()

    const = ctx.enter_context(tc.tile_pool(name="const", bufs=1))
    state = ctx.enter_context(tc.tile_pool(name="state", bufs=1))
    xpool = ctx.enter_context(tc.tile_pool(name="x", bufs=3))
    qpool = ctx.enter_context(tc.tile_pool(name="q", bufs=2))
    zpool = ctx.enter_context(tc.tile_pool(name="z", bufs=2))
    wpool = ctx.enter_context(tc.tile_pool(name="wv", bufs=2))
    vpool = ctx.enter_context(tc.tile_pool(name="vblk", bufs=2))
    spool = ctx.enter_context(tc.tile_pool(name="sblk", bufs=2))
    pp1 = ctx.enter_context(tc.tile_pool(name="ps1", bufs=3, space="PSUM"))

    w1sb = const.tile([P, PAIRS * 2 * P], f8)
    nc.sync.dma_start(w1sb[:], w1)
    w2sb = const.tile([H, OW], f16)
    nc.sync.dma_start(w2sb[:], w2)
    nIsb = const.tile([FH, FH], f16)
    nc.sync.dma_start(nIsb[:], nI)
    gsb = const.tile([FH, 1], f32)
    nc.sync.dma_start(gsb[:], g)
    cc = const.tile([FH, BC], f16)
    nc.vector.memset(cc[:], C_)
    b15 = const.tile([FH, 1], f32)
    nc.vector.memset(b15[:], 15.0)
    v0 = const.tile([FH, BC], f16)
    nc.vector.memset(v0[:], -70.0)

    u = state.tile([FH, BC], f16)
    nc.sync.dma_start(u[:], ui)

    vprev = v0[:]
    spk_prev = None
    cur_v = cur_s = None
    for i in range(NIT):
        vb, col = divmod(i, FLUSH)
        if col == 0:
            cur_v = vpool.tile([FH, FLUSH * BC], f16, tag="vblk")
            cur_s = spool.tile([FH, FLUSH * BC], f16, tag="sblk")
        svcol = cur_v[:, col * BC:(col + 1) * BC]
        sscol = cur_s[:, col * BC:(col + 1) * BC]

        lo = 0 if i < T else 96     # active fused rows [lo:hi]
        hi = FH if i >= 1 else H
        R = slice(lo, hi)

        if i == 0:
            # layer-2 rows of the first column are read as v_prev next iter
            nc.vector.memset(svcol[96:FH], -70.0)

        # ---- matmuls: one fused PSUM tile.  W2 matmul writes rows 96:110
        # first (zero-padded stationary puts cur2 at 100:110), then the
        # layer-1 group overwrites rows 0:100 (start=True resets them), and
        # a -I matmul accumulates -U so pf = cur - U directly. ----
        pf = pp1.tile([FH, BC], f32)
        if i >= 1:
            nc.tensor.matmul(pf[96:FH, :], w2sb[:], spk_prev,
                             start=True, stop=True, tile_position=(0, 96),
                             skip_group_check=True)
        if i < T:
            tb, tt = divmod(i, TB)
            if tt == 0:
                xt = xpool.tile([P, TB * PAIRS * 2 * BC], f8)
                nc.sync.dma_start(xt[:], xT[tb, :, :])
            for j in range(PAIRS):
                o0 = ((tt * PAIRS + j) * 2) * BC
                rhs = xt[:, o0:o0 + 2 * BC].rearrange("p (s b) -> p s b", s=2)
                lhsT = w1sb[:, j * 2 * P:(j + 1) * 2 * P].rearrange(
                    "p (s h) -> p s h", s=2)[:, :, 0:H]
                nc.tensor.matmul(pf[0:H, :], lhsT, rhs,
                                 start=(j == 0), stop=(j == PAIRS - 1),
                                 perf_mode=DR)
        if i < T:
            ni = FH if i >= 1 else H
            nc.tensor.matmul(pf[0:ni, :], nIsb[0:ni, 0:ni], u[0:ni, :],
                             start=False, stop=True, skip_group_check=True)

        # ---- activations (from pre-update v); z on Pool ----
        q = qpool.tile([FH, BC], f16, tag="q")
        z = zpool.tile([FH, BC], f16, tag="z")
        nc.scalar.activation(q[R], vprev[R], AF.Square, bias=b15[R, 0:1],
                             scale=0.2)
        nc.gpsimd.tensor_scalar(z[R], vprev[R], A_ * B_, gsb[R, 0:1],
                                AL.mult, AL.add)

        # ---- izhikevich update on fused rows (pf already holds cur - U;
        # at i=100 no -I matmul ran, so subtract U explicitly) ----
        if i == T:
            wv = wpool.tile([FH, BC], f16, tag="wv")
            nc.vector.scalar_tensor_tensor(wv[R], pf[R], 1.0, u[R],
                                           AL.mult, AL.subtract)
            nc.vector.tensor_tensor(svcol[R], q[R], wv[R], AL.add)
        else:
            nc.vector.tensor_tensor(svcol[R], q[R], pf[R], AL.add)
        nc.vector.tensor_scalar(sscol[R], svcol[R], THR, None, AL.is_ge)
        nc.vector.copy_predicated(svcol[R],
                                  sscol[R].bitcast(mybir.dt.uint16), cc[R])
        nc.vector.scalar_tensor_tensor(u[R], u[R], 1.0 - A_, z[R],
                                       AL.mult, AL.add)
        nc.vector.scalar_tensor_tensor(u[R], sscol[R], D_, u[R],
                                       AL.mult, AL.add)

        # ---- stream outputs once per block (rows 96:110; host drops 96:100)
        if col == FLUSH - 1 or i == NIT - 1:
            c0 = 1 if vb == 0 else 0
            n = col + 1 - c0
            t0 = vb * FLUSH + c0 - 1
            nc.sync.dma_start(
                out_s[:, t0:t0 + n, :],
                cur_s[96:FH, c0 * BC:(col + 1) * BC].rearrange(
                    "p (t b) -> p t b", t=n))
            nc.sync.dma_start(
                out_m[:, t0:t0 + n, :],
                cur_v[96:FH, c0 * BC:(col + 1) * BC].rearrange(
                    "p (t b) -> p t b", t=n))

        vprev = svcol
        spk_prev = sscol[0:H, :]


def _host_inputs(x, W1, b1, W2, b2):
    """Per-core input dicts. x: [BATCH, T, F] fp32."""
    f8 = ml_dtypes.float8_e4m3
    W1p = np.zeros((H, FPAD), np.float32)
    W1p[:, :F] = W1
    # w1t[p, j, s, 0:100] = W1[h, 224 j + 112 s + p]; 112-col pitch per block
    w1t = np.zeros((P, PAIRS, 2, P), np.float32)
    w1t[:, :, :, 0:H] = W1p.reshape(H, PAIRS, 2, P).transpose(3, 1, 2, 0)
    w1t = np.ascontiguousarray(w1t).reshape(P, PAIRS * 2 * P).astype(f8)
    w2t = np.zeros((H, O + 4), np.float16)
    w2t[:, 4:] = W2.T.astype(np.float16)
    nI = (-np.eye(FH, dtype=np.float32)).astype(np.float16)
    beta = np.concatenate([b1, b2])  # [110]
    ui = np.ascontiguousarray(
        np.broadcast_to((70.0 - beta)[:, None], (FH, BC))).astype(np.float16)
    g = np.ascontiguousarray((A_ * (85.0 - beta))[:, None].astype(np.float32))
    n_cores = x.shape[0] // BC
    in_maps = []
    for i in range(n_cores):
        xs = x[i * BC:(i + 1) * BC]  # [BC, T, F]
        xp = np.zeros((BC, T, FPAD), np.float32)
        xp[:, :, :F] = xs
        # xT[tb, p, (tt, j, s, b)] = x[b, 2 tb + tt, 224 j + 112 s + p]
        xTi = xp.reshape(BC, T // TB, TB, PAIRS, 2, P).transpose(
            1, 5, 2, 3, 4, 0).astype(f8).reshape(T // TB, P, TB * PAIRS * 2 * BC)
        xTi = np.ascontiguousarray(xTi)
        in_maps.append({
            "xT": xTi, "w1t": w1t, "w2t": w2t, "nI": nI, "ui": ui, "g": g,
        })
    return in_maps


def _install_ntff_shim():
    """Register the NTFF profile hook when the image's antenv lacks axon_hooks.

    Only needed for BASS_TRACE profiling runs; silently a no-op if anything
    is missing so plain correctness runs never depend on it.
    """
    import sys
    import types
    try:
        import antenv.axon_hooks  # noqa: F401  # already present: nothing to do
        return
    except ImportError:
        pass
    try:
        from trn_agent_boot.trn_boot import _ntff_profile_via_ctypes
        hook = _ntff_profile_via_ctypes("/opt/axon/libaxon_pjrt.so")
        mod = types.ModuleType("antenv.axon_hooks")
        mod._hook = hook
        mod.get_axon_ntff_profile_hook = lambda: mod._hook
        mod.set_axon_ntff_profile_hook = lambda h: setattr(mod, "_hook", h)
        sys.modules["antenv.axon_hooks"] = mod
    except Exception:
        pass


def kernel(x, W1, b1, W2, b2):
    global LAST_RUN
    if os.environ.get("BASS_TRACE"):
        _install_ntff_shim()
    x = np.ascontiguousarray(x, dtype=np.float32)
    W1 = np.asarray(W1, np.float32)
    b1 = np.asarray(b1, np.float32)
    W2 = np.asarray(W2, np.float32)
    b2 = np.asarray(b2, np.float32)

    nc = bacc.Bacc("TRN2", target_bir_lowering=False, debug=False,
                   num_devices=NCORES)
    with tile.TileContext(nc) as tc:
        with ExitStack() as ctx:
            build_program(nc, ctx, tc)
    nc.compile()

    in_maps = _host_inputs(x, W1, b1, W2, b2)
    res = run_bass_kernel_spmd(
        nc, in_maps, core_ids=list(range(NCORES)),
        trace=bool(os.environ.get("BASS_TRACE")),
    )
    LAST_RUN = res

    spk = np.empty((T, BATCH, O), np.float32)
    mem = np.empty((T, BATCH, O), np.float32)
    for i in range(NCORES):
        r = res.results[i]
        spk[:, i * BC:(i + 1) * BC, :] = \
            r["out_s"][4:, :, :].astype(np.float32).transpose(1, 2, 0)
        mem[:, i * BC:(i + 1) * BC, :] = \
            r["out_m"][4:, :, :].astype(np.float32).transpose(1, 2, 0)
    return spk, mem
